# revision 1
# baseline (speedup 1.0000x reference)
"""Contrastive loss kernel for Trainium2 (8 NeuronCores, Bass/Tile).

Strategy
--------
Only rows with label==1 (pos) contribute losses, and only columns with
label==0 (neg) plus the diagonal enter each row's logsumexp.  The host
computes the tiny index sets from `labels`, then each of the 8 cores
(2 per batch) receives:
  gp: its half of the batch's positive greek rows      [P1, 256] f32
  ep: english rows at the same indices (for the diag)  [P1, 256] f32
  en: all negative english rows of the batch           [N1, 256] f32
padded with zero rows to the uniform compile-time shapes (P1, N1).

On device: L2-normalize rows (1/temperature folded into the greek
scale), cast bf16, PE-transpose to put H on partitions, matmul to get
logits in PSUM, then a single fused ScalarE pass exp(logit - 15) with
accumulate gives the per-row negative sums.  A fixed max constant (15 >
1/0.07) replaces the per-row max: logits are bounded so the logsumexp
stays exact in f32.  Zero-padded `en` rows yield *exactly* 0 logits, so
their exp(-15) contributions are removed with an exact scalar
correction.  Per-row loss = 15 + ln(exp(diag-15) + S + corr) - diag,
masked by a 0/1 weight vector and row-reduced; the host sums the 8x128
partials and divides by the positive count.
"""

import sys

if "/opt/trn_rl_repo" not in sys.path:
    sys.path.insert(0, "/opt/trn_rl_repo")

from contextlib import ExitStack

import ml_dtypes
import numpy as np

import concourse.bass as bass
import concourse.tile as tile
from concourse import mybir
from concourse.bass_utils import run_bass_kernel_spmd
from concourse.masks import make_identity

TEMPERATURE = 0.07
IGNORE_INDEX = -100
CMAX = 15.0
H = 256
N_CORES = 8

# Stash of the most recent BassKernelResults + shapes (for test harness timing).
LAST_RESULTS = None
LAST_SHAPES = None
TRACE = False


def _legalize_waits(nc: bass.Bass, max_waits: int = 1) -> None:
    """This container's walrus accepts at most one sync-wait per instruction
    (ACT structs especially); Tile can emit several.  Split the excess onto
    same-engine NoOps placed immediately before the instruction."""
    for bb in nc.main_func.blocks:
        new = []
        for ins in bb.instructions:
            si = ins.sync_info
            if si is not None and si.on_wait and len(si.on_wait) > max_waits:
                waits = list(si.on_wait)
                extra, keep = waits[:-max_waits], waits[-max_waits:]
                for i in range(0, len(extra), max_waits):
                    new.append(
                        mybir.InstNoOp(
                            name=nc.get_next_instruction_name(),
                            engine=ins.engine,
                            ins=[],
                            outs=[],
                            sync_info=mybir.SyncInfo(
                                on_wait=extra[i : i + max_waits], on_update=[]
                            ),
                            bass_nofuse=True,
                        )
                    )
                ins.sync_info = mybir.SyncInfo(
                    on_wait=keep, on_update=list(si.on_update or [])
                )
            new.append(ins)
        bb.instructions[:] = new


def _build_program(P1: int, N1: int, legalize: bool = True) -> bass.Bass:
    """One SPMD program: shapes P1 (pos rows) and N1 (neg rows) are uniform
    across cores; per-core data differs via in_maps."""
    PC = P1 // 128
    NC = N1 // 128
    NTILES = N1 // 512
    GROUPS = NC // 4  # 4-chunk transpose groups == 512-wide matmul slabs
    f32 = mybir.dt.float32
    bf16 = mybir.dt.bfloat16
    OP = mybir.AluOpType
    AF = mybir.ActivationFunctionType

    nc = bass.Bass()
    gp = nc.dram_tensor("gp", [P1, H], bf16, kind="ExternalInput")
    ep = nc.dram_tensor("ep", [P1, H], bf16, kind="ExternalInput")
    en = nc.dram_tensor("en", [N1, H], bf16, kind="ExternalInput")
    wv = nc.dram_tensor("wv", [P1], f32, kind="ExternalInput")
    corr = nc.dram_tensor("corr", [1, 1], f32, kind="ExternalInput")
    out = nc.dram_tensor("out", [128, 1], f32, kind="ExternalOutput")

    with tile.TileContext(nc) as tc, ExitStack() as ctx:
        persist = ctx.enter_context(tc.tile_pool(name="persist", bufs=1))
        small = ctx.enter_context(tc.tile_pool(name="small", bufs=1))
        scratch = ctx.enter_context(tc.tile_pool(name="scratch", bufs=3))
        expool = ctx.enter_context(tc.tile_pool(name="expool", bufs=2))
        psum_tp = ctx.enter_context(tc.tile_pool(name="psum_tp", bufs=2, space="PSUM"))
        psum_mm = ctx.enter_context(tc.tile_pool(name="psum_mm", bufs=2, space="PSUM"))

        # ---- constants (gpsimd: otherwise idle) + ACT table preload
        LOG_INV_T = float(-np.log(np.float64(TEMPERATURE)))
        eps_t = small.tile([128, 1], f32)
        nc.gpsimd.memset(eps_t[:], 1e-24)
        blnt_t = small.tile([128, 1], f32)
        nc.gpsimd.memset(blnt_t[:], LOG_INV_T)
        cneg_t = small.tile([128, 1], f32)
        nc.gpsimd.memset(cneg_t[:], -CMAX)
        ident = small.tile([128, 128], bf16)
        make_identity(nc, ident[:])
        # Dummy Ln at t~0 absorbs the ~2.7us ACT table load during the DMAs.
        dummy = small.tile([128, 1], f32)
        nc.scalar.activation(
            out=dummy[:], in_=eps_t[:], func=AF.Ln, bias=eps_t[:, 0:1], scale=1.0
        )

        # ---- loads (bf16), split per 4-chunk piece across the DMA queues
        # (SP + ACT hardware DGE, gpsimd software DGE) so they run in
        # parallel and unblock the pipeline piece by piece.
        # partition i holds rows {c*128+i : c in range(chunks)}
        Gf = persist.tile([128, PC, H], bf16)
        nc.sync.dma_start(out=Gf[:], in_=gp[:].rearrange("(c p) h -> p c h", p=128))
        en_r = en[:].rearrange("(c p) h -> p c h", p=128)
        Np = []
        for g in range(GROUPS):
            t = persist.tile([128, 4, H], bf16, tag=f"np{g}", name=f"np{g}")
            eng = nc.scalar if g % 2 == 1 else nc.sync
            eng.dma_start(out=t[:], in_=en_r[:, g * 4 : (g + 1) * 4, :])
            Np.append(t)
        Ef = persist.tile([128, PC, H], bf16)
        nc.gpsimd.dma_start(out=Ef[:], in_=ep[:].rearrange("(c p) h -> p c h", p=128))
        wt = small.tile([128, PC], f32)
        nc.sync.dma_start(out=wt[:], in_=wv[:].rearrange("(c p) -> p c", p=128))
        corr_t = small.tile([128, 1], f32)
        nc.sync.dma_start(out=corr_t[:], in_=corr[:].to_broadcast([128, 1]))

        # ---- row sums of squares (per 128-row chunk), piece-granular for e
        ssn = []
        for g in range(GROUPS):
            t = small.tile([128, 4], f32, tag=f"ssn{g}", name=f"ssn{g}")
            ssn.append(t)
        ssg = small.tile([128, PC], f32)
        sse = small.tile([128, PC], f32)

        def norm_jobs(xf, c, ss, sc):
            sq = scratch.tile([128, H], bf16, tag="sq")
            nc.vector.scalar_tensor_tensor(
                out=sq[:],
                in0=xf[:, c, :],
                scalar=1.0,
                in1=xf[:, c, :],
                op0=OP.mult,
                op1=OP.mult,
                accum_out=ss[:, sc : sc + 1],
            )

        def scale_of(ss, b):
            # rsqrt as exp(-0.5*ln(ss+eps)): one ACT table set for ln+exp.
            # eps=1e-24 matches the reference's clip(norm, 1e-12).
            nc.scalar.activation(
                out=ss[:], in_=ss[:], func=AF.Ln, bias=eps_t[:, 0:1], scale=1.0
            )
            bias = b if isinstance(b, float) else b[:, 0:1]
            nc.scalar.activation(out=ss[:], in_=ss[:], func=AF.Exp, bias=bias, scale=-0.5)

        # greek norms first (its chain ends at the matmul stationary side),
        # then the e pieces in arrival order
        for c in range(PC):
            norm_jobs(Gf, c, ssg, c)
        scale_of(ssg, blnt_t)  # greek scale carries the 1/T
        for g in range(GROUPS):
            for c in range(4):
                norm_jobs(Np[g], c, ssn[g], c)
            scale_of(ssn[g], 0.0)

        # ---- apply scales -> bf16 matmul operands, on the idle gpsimd
        Gb = persist.tile([128, PC, H], bf16)
        for c in range(PC):
            nc.gpsimd.tensor_scalar_mul(Gb[:, c, :], Gf[:, c, :], ssg[:, c : c + 1])
        Nb = []
        for g in range(GROUPS):
            t = persist.tile([128, 4, H], bf16, tag=f"nb{g}", name=f"nb{g}")
            for c in range(4):
                nc.gpsimd.tensor_scalar_mul(t[:, c, :], Np[g][:, c, :], ssn[g][:, c : c + 1])
            Nb.append(t)

        # ---- transpose to put H on partitions (PE) + copy PSUM->SBUF (DVE)
        GbT = persist.tile([128, 2, P1], bf16)
        for c0 in range(0, PC, 4):
            cn = min(4, PC - c0)
            for hk in range(2):
                pt = psum_tp.tile([128, 512], bf16, tag="pt")
                for j in range(cn):
                    nc.tensor.transpose(
                        pt[:, j * 128 : (j + 1) * 128],
                        Gb[:, c0 + j, hk * 128 : (hk + 1) * 128],
                        ident[:],
                    )
                nc.scalar.copy(
                    out=GbT[:, hk, c0 * 128 : (c0 + cn) * 128], in_=pt[:, : cn * 128]
                )
        NbT = [
            persist.tile([128, 2, 512], bf16, tag=f"nbt{g}", name=f"nbt{g}")
            for g in range(GROUPS)
        ]
        for g in range(GROUPS):
            for hk in range(2):
                pt = psum_tp.tile([128, 512], bf16, tag="pt")
                for j in range(4):
                    nc.tensor.transpose(
                        pt[:, j * 128 : (j + 1) * 128],
                        Nb[g][:, j, hk * 128 : (hk + 1) * 128],
                        ident[:],
                    )
                nc.vector.tensor_copy(out=NbT[g][:, hk, :], in_=pt[:])

        # ---- logits + one fused exp/accumulate pass per 128-row chunk
        # S[p, c] = sum_q exp(logit[c*128+p, q] - CMAX)
        S = small.tile([128, PC], f32)
        for c in range(PC):
            pm = psum_mm.tile([128, N1], f32, tag="pm")
            for nt in range(NTILES):
                for hk in range(2):
                    nc.tensor.matmul(
                        pm[:, nt * 512 : (nt + 1) * 512],
                        GbT[:, hk, c * 128 : (c + 1) * 128],
                        NbT[nt][:, hk, :],
                        start=(hk == 0),
                        stop=(hk == 1),
                    )
            ex = expool.tile([128, N1], f32, tag="ex")
            nc.scalar.activation(
                out=ex[:],
                in_=pm[:],
                func=AF.Exp,
                bias=cneg_t[:, 0:1],
                scale=1.0,
                accum_out=S[:, c : c + 1],
            )

        # ---- diag[p] = raw greek.english dot, scaled by both row norms
        for c in range(PC):
            norm_jobs(Ef, c, sse, c)
        scale_of(sse, 0.0)
        diag = small.tile([128, PC], f32)
        for c in range(PC):
            dsq = scratch.tile([128, H], bf16, tag="dsq")
            nc.vector.scalar_tensor_tensor(
                out=dsq[:],
                in0=Gf[:, c, :],
                scalar=1.0,
                in1=Ef[:, c, :],
                op0=OP.mult,
                op1=OP.mult,
                accum_out=diag[:, c : c + 1],
            )
        nc.vector.tensor_mul(diag[:], diag[:], ssg[:])
        nc.vector.tensor_mul(diag[:], diag[:], sse[:])

        # ---- per-row loss and masked partial sum
        ed = small.tile([128, PC], f32)
        nc.scalar.activation(
            out=ed[:], in_=diag[:], func=AF.Exp, bias=cneg_t[:, 0:1], scale=1.0
        )
        t2 = small.tile([128, PC], f32)
        nc.vector.scalar_tensor_tensor(
            out=t2[:],
            in0=S[:],
            scalar=corr_t[:, 0:1],
            in1=ed[:],
            op0=OP.add,
            op1=OP.add,
        )
        nc.scalar.activation(out=t2[:], in_=t2[:], func=AF.Ln)
        # loss = (ln(...) + CMAX) - diag
        loss = small.tile([128, PC], f32)
        nc.vector.scalar_tensor_tensor(
            out=loss[:],
            in0=t2[:],
            scalar=CMAX,
            in1=diag[:],
            op0=OP.add,
            op1=OP.subtract,
        )
        lm = small.tile([128, PC], f32)
        part = small.tile([128, 1], f32)
        nc.vector.scalar_tensor_tensor(
            out=lm[:],
            in0=loss[:],
            scalar=1.0,
            in1=wt[:],
            op0=OP.mult,
            op1=OP.mult,
            accum_out=part[:],
        )
        nc.sync.dma_start(out=out[:], in_=part[:])
    if legalize:
        _legalize_waits(nc, max_waits=1)
    return nc


def _pad_rows(x: np.ndarray, n: int) -> np.ndarray:
    outp = np.zeros((n,) + x.shape[1:], dtype=x.dtype)
    outp[: x.shape[0]] = x
    return outp


def kernel(greek_embeds, english_embeds, labels):
    global LAST_RESULTS
    g = np.ascontiguousarray(np.asarray(greek_embeds, dtype=np.float32))
    e = np.ascontiguousarray(np.asarray(english_embeds, dtype=np.float32))
    lab = np.asarray(labels)
    B, P, Hh = g.shape
    assert Hh == H and B * 2 == N_CORES

    valid = lab != IGNORE_INDEX
    pos = valid & (lab == 1)
    neg = valid & (lab != 1)
    ok = (valid.sum(-1) >= 2) & pos.any(-1) & neg.any(-1)

    count = int(pos[ok].sum()) if ok.any() else 0
    if count == 0:
        return np.float32(0.0)

    pos_idx = [np.nonzero(pos[b])[0] if ok[b] else np.zeros(0, np.int64) for b in range(B)]
    neg_idx = [np.nonzero(neg[b])[0] if ok[b] else np.zeros(0, np.int64) for b in range(B)]
    halves = [np.array_split(pi, 2) for pi in pos_idx]

    np_max = max(len(halves[b][h]) for b in range(B) for h in range(2))
    nn_max = max(len(ni) for ni in neg_idx)
    P1 = max(128, ((np_max + 127) // 128) * 128)
    N1 = max(512, ((nn_max + 511) // 512) * 512)

    E15 = np.float32(np.exp(np.float32(-CMAX)))
    in_maps = []
    for core in range(N_CORES):
        b, hf = core // 2, core % 2
        p_idx = halves[b][hf]
        n_idx = neg_idx[b]
        w = np.zeros(P1, np.float32)
        w[: len(p_idx)] = 1.0
        in_maps.append(
            {
                "gp": _pad_rows(g[b][p_idx].astype(ml_dtypes.bfloat16), P1),
                "ep": _pad_rows(e[b][p_idx].astype(ml_dtypes.bfloat16), P1),
                "en": _pad_rows(e[b][n_idx].astype(ml_dtypes.bfloat16), N1),
                "wv": w,
                "corr": np.array([[-(N1 - len(n_idx)) * float(E15)]], np.float32),
            }
        )

    global LAST_SHAPES
    LAST_SHAPES = (P1, N1, dict(in_maps[0]))
    nc = _build_program(P1, N1)
    res = run_bass_kernel_spmd(nc, in_maps, list(range(N_CORES)), trace=TRACE)
    LAST_RESULTS = res
    total = sum(float(r["out"].sum()) for r in res.results)
    return np.float32(total / count)



# revision 12
# speedup vs baseline: 1.0523x; 1.0523x over previous
"""Contrastive loss kernel for Trainium2 (8 NeuronCores, Bass/Tile).

Strategy
--------
Only rows with label==1 (pos) contribute losses, and only columns with
label==0 (neg) plus the diagonal enter each row's logsumexp.  The host
computes the tiny index sets from `labels`, then each of the 8 cores
(2 per batch) receives:
  gp : its half of the batch's positive greek rows, row-major [P1,256]
  gpt: the same rows pre-transposed on host [2,128,P1] (H on partitions)
  ep : english rows at the same indices (diag term)    [P1,256]
  en : all negative english rows of the batch          [N1,256]
all bf16, zero-padded to uniform compile-time shapes (P1, N1).

Device pipeline: row sums-of-squares via fused square+accumulate ops
split between DVE (STT) and the Scalar engine (Square activation with
accumulate, which shares the Ln/Exp table so no reload); inverse norms
via batched Ln / Exp(scale=-0.5) pairs.  The greek scale (with 1/T
folded via a T^2 factor in its squares) is applied as the per-partition
`scale` operand of the exp pass, so raw host-transposed gpt feeds the
matmul directly.  English rows are scaled with 4x-mode tensor_scalar
ops (DVE/Pool split), PE-transposed, and copied to SBUF.  Logits
accumulate in PSUM (bf16 matmul, full width per 128-row pos chunk); one
in-place exp(s_g*x - 15) pass with accumulate per chunk produces the
negative sums.  Zero-padded en rows give exactly 0 logits; their
exp(-15) mass is removed by an exact host-computed correction.  The
diag term runs entirely in the slack during the exp phase.  Per-row
loss splits into wv*ln(S+corr+e^(d-15)) + wv*(15-d); both partial sums
are reduced on-device and the host sums 8x128x2 partials / count.
"""

import sys

if "/opt/trn_rl_repo" not in sys.path:
    sys.path.insert(0, "/opt/trn_rl_repo")

from contextlib import ExitStack

import ml_dtypes
import numpy as np

import concourse.bass as bass
import concourse.tile as tile
from concourse import mybir
from concourse.bass_utils import run_bass_kernel_spmd
from concourse.masks import make_identity

TEMPERATURE = 0.07
IGNORE_INDEX = -100
CMAX = 15.0
H = 256
N_CORES = 8

# Stash of the most recent BassKernelResults + shapes (for test harness timing).
LAST_RESULTS = None
LAST_SHAPES = None
TRACE = False


def _legalize_waits(nc: bass.Bass, max_waits: int = 1) -> None:
    """This container's walrus accepts at most one sync-wait per instruction
    (ACT structs especially); Tile can emit several.  Split the excess onto
    same-engine NoOps placed immediately before the instruction."""
    for bb in nc.main_func.blocks:
        new = []
        for ins in bb.instructions:
            si = ins.sync_info
            if si is not None and si.on_wait and len(si.on_wait) > max_waits:
                waits = list(si.on_wait)
                extra, keep = waits[:-max_waits], waits[-max_waits:]
                for i in range(0, len(extra), max_waits):
                    new.append(
                        mybir.InstNoOp(
                            name=nc.get_next_instruction_name(),
                            engine=ins.engine,
                            ins=[],
                            outs=[],
                            sync_info=mybir.SyncInfo(
                                on_wait=extra[i : i + max_waits], on_update=[]
                            ),
                            bass_nofuse=True,
                        )
                    )
                ins.sync_info = mybir.SyncInfo(
                    on_wait=keep, on_update=list(si.on_update or [])
                )
            new.append(ins)
        bb.instructions[:] = new


def _build_program(P1: int, N1: int, legalize: bool = True) -> bass.Bass:
    """One SPMD program: shapes P1 (pos rows) and N1 (neg rows) are uniform
    across cores; per-core data differs via in_maps."""
    PC = P1 // 128
    NC = N1 // 128
    assert N1 <= 1664 and P1 <= 1664
    f32 = mybir.dt.float32
    bf16 = mybir.dt.bfloat16
    OP = mybir.AluOpType
    AF = mybir.ActivationFunctionType

    # en DMA pieces across the SP / ACT / Pool queues
    a = min(5, NC)
    b = min(a + 5, NC)
    pieces = [(0, a), (a, b), (b, NC)]
    # which engine squares each en chunk: DVE except a couple on ACT
    act_sq_en = set(range(a, min(a + 2, b)))
    # greek squares: first chunks on ACT (idle window), rest DVE
    act_sq_g = set(range(0, min(2, PC)))
    # 512-wide matmul tiles over N1
    nts = []
    c0 = 0
    while c0 < N1:
        w = min(512, N1 - c0)
        nts.append((c0, w))
        c0 += w

    nc = bass.Bass()
    gp = nc.dram_tensor("gp", [P1, H], bf16, kind="ExternalInput")
    gpt = nc.dram_tensor("gpt", [2, 128, P1], bf16, kind="ExternalInput")
    ep = nc.dram_tensor("ep", [P1, H], bf16, kind="ExternalInput")
    en = nc.dram_tensor("en", [N1, H], bf16, kind="ExternalInput")
    wv = nc.dram_tensor("wv", [P1], f32, kind="ExternalInput")
    corr = nc.dram_tensor("corr", [1, 1], f32, kind="ExternalInput")
    out = nc.dram_tensor("out", [128, 2], f32, kind="ExternalOutput")

    with tile.TileContext(nc) as tc, ExitStack() as ctx:
        persist = ctx.enter_context(tc.tile_pool(name="persist", bufs=1))
        small = ctx.enter_context(tc.tile_pool(name="small", bufs=1))
        scratch = ctx.enter_context(tc.tile_pool(name="scratch", bufs=4))
        psum_tp = ctx.enter_context(tc.tile_pool(name="psum_tp", bufs=2, space="PSUM"))
        psum_mm = ctx.enter_context(tc.tile_pool(name="psum_mm", bufs=2, space="PSUM"))

        # ---- constants (gpsimd) -----------------------------------------
        cneg_t = small.tile([128, 1], f32)
        nc.gpsimd.memset(cneg_t[:], -CMAX)
        eps_t = small.tile([128, 1], f32)
        nc.gpsimd.memset(eps_t[:], 1e-24)
        ident = small.tile([128, 128], bf16)
        make_identity(nc, ident[:])

        # ---- DMA loads on parallel queues --------------------------------
        # partition i of chunk c holds row c*128+i
        en_r = en[:].rearrange("(c p) h -> p c h", p=128)
        EnP = []
        for qi, (lo, hi) in enumerate(pieces):
            if hi <= lo:
                EnP.append(None)
                continue
            t = persist.tile([128, hi - lo, H], bf16, tag=f"en{qi}", name=f"en{qi}")
            eng = [nc.sync, nc.scalar, nc.gpsimd][qi]
            eng.dma_start(out=t[:], in_=en_r[:, lo:hi, :])
            EnP.append(t)

        def en_chunk(c):
            for (lo, hi), t in zip(pieces, EnP):
                if lo <= c < hi:
                    return t[:, c - lo, :]
            raise AssertionError

        Gf = persist.tile([128, PC, H], bf16)
        nc.sync.dma_start(out=Gf[:], in_=gp[:].rearrange("(c p) h -> p c h", p=128))
        GT = persist.tile([128, 2, P1], bf16)
        nc.sync.dma_start(out=GT[:], in_=gpt[:].rearrange("k p j -> p k j"))
        Ef = persist.tile([128, PC, H], bf16)
        nc.gpsimd.dma_start(out=Ef[:], in_=ep[:].rearrange("(c p) h -> p c h", p=128))
        wt = small.tile([128, PC], f32)
        nc.sync.dma_start(out=wt[:], in_=wv[:].rearrange("(c p) -> p c", p=128))
        corr_t = small.tile([128, 1], f32)
        nc.sync.dma_start(out=corr_t[:], in_=corr[:].to_broadcast([128, 1]))

        # ---- PE warmup: anchors the p-state ramp so the logit matmuls
        # (starting ~3.5us in) run at full clock.
        ptw = psum_tp.tile([128, 512], bf16, tag="pt")
        nc.tensor.transpose(ptw[:, 0:128], ident[:], ident[:])

        # ---- ACT table preload (Ln/Exp/Square share one set); overlaps DMA.
        dummy = small.tile([128, 1], f32)
        nc.scalar.activation(
            out=dummy[:], in_=eps_t[:], func=AF.Ln, bias=eps_t[:, 0:1], scale=1.0
        )

        # ---- row sums of squares ----------------------------------------
        # ss[:, 0:NC] = en ; ss[:, NC:NC+PC] = T^2 * g  (T folds 1/temp into
        # the greek scale) ; ss2[:, 0:PC] = ep
        ss = small.tile([128, NC + PC], f32)
        ss2 = small.tile([128, PC], f32)

        def sq_dve(src, ss_t, col, scalar=1.0):
            sq = scratch.tile([128, H], bf16, tag="sq")
            nc.vector.scalar_tensor_tensor(
                out=sq[:], in0=src, scalar=scalar, in1=src,
                op0=OP.mult, op1=OP.mult,
                accum_out=ss_t[:, col : col + 1],
            )

        def sq_act(src, ss_t, col, scale=1.0):
            sq = scratch.tile([128, H], bf16, tag="sq")
            nc.scalar.activation(
                out=sq[:], in_=src, func=AF.Square, bias=0.0, scale=scale,
                accum_out=ss_t[:, col : col + 1],
            )

        for c in range(NC):
            if c in act_sq_en:
                sq_act(en_chunk(c), ss, c)
            else:
                sq_dve(en_chunk(c), ss, c)
        for c in range(PC):
            if c in act_sq_g:
                sq_act(Gf[:, c, :], ss, NC + c, scale=float(TEMPERATURE))
            else:
                sq_dve(Gf[:, c, :], ss, NC + c, scalar=float(TEMPERATURE**2))

        # ---- inverse norms: s = (ss+eps)^-0.5 via Ln + Exp(scale=-0.5)
        s = small.tile([128, NC + PC], f32)
        nc.scalar.activation(
            out=s[:, 0:NC], in_=ss[:, 0:NC], func=AF.Ln, bias=eps_t[:, 0:1], scale=1.0
        )
        nc.scalar.activation(
            out=s[:, 0:NC], in_=s[:, 0:NC], func=AF.Exp, bias=0.0, scale=-0.5
        )
        nc.scalar.activation(
            out=s[:, NC:], in_=ss[:, NC:], func=AF.Ln, bias=eps_t[:, 0:1], scale=1.0
        )
        nc.scalar.activation(
            out=s[:, NC:], in_=s[:, NC:], func=AF.Exp, bias=0.0, scale=-0.5
        )

        # ---- scale en rows: Enb[q] = en[q] * s_q (DVE 4x-mode / Pool split)
        Enb = persist.tile([128, NC, H], bf16)
        for c in range(NC):
            eng = nc.vector if c % 2 == 0 else nc.gpsimd
            eng.tensor_scalar_mul(Enb[:, c, :], en_chunk(c), s[:, c : c + 1])

        # ---- transpose: NbT[:, hk, c*128+q] = Enb[q of chunk c, hk*128+h]
        NbT = persist.tile([128, 2, N1], bf16)
        for g0 in range(0, NC, 4):
            gn = min(4, NC - g0)
            for hk in range(2):
                pt = psum_tp.tile([128, gn * 128], bf16, tag="pt")
                for j in range(gn):
                    nc.tensor.transpose(
                        pt[:, j * 128 : (j + 1) * 128],
                        Enb[:, g0 + j, hk * 128 : (hk + 1) * 128],
                        ident[:],
                    )
                nc.vector.tensor_copy(
                    out=NbT[:, hk, g0 * 128 : (g0 + gn) * 128], in_=pt[:]
                )

        # ---- diag path emission (slack work; runs during the exp phase) --
        # ep squares + dots on DVE; rsqrt pair inserted between exps below.
        dot = small.tile([128, PC], f32)
        for cc in range(PC):
            sq_dve(Ef[:, cc, :], ss2, cc)
            dsq = scratch.tile([128, H], bf16, tag="sq")
            nc.vector.scalar_tensor_tensor(
                out=dsq[:], in0=Gf[:, cc, :], scalar=1.0, in1=Ef[:, cc, :],
                op0=OP.mult, op1=OP.mult,
                accum_out=dot[:, cc : cc + 1],
            )
        se = small.tile([128, PC], f32)
        diagn = small.tile([128, PC], f32)
        ed = small.tile([128, PC], f32)
        v2 = small.tile([128, PC], f32)
        pr = small.tile([128, 2], f32)
        v2m = small.tile([128, PC], f32)

        def emit_diag_tail():
            # after se = rsqrt(ss2) is ready (DVE ops)
            nc.vector.tensor_tensor(
                out=diagn[:], in0=dot[:], in1=s[:, NC:], op=OP.mult
            )
            nc.vector.tensor_tensor(
                out=diagn[:], in0=diagn[:], in1=se[:], op=OP.mult
            )
            # v2 = 15 - diagn ; pv = sum(wv * v2)
            nc.vector.tensor_scalar(
                out=v2[:], in0=diagn[:], scalar1=-1.0, scalar2=CMAX,
                op0=OP.mult, op1=OP.add,
            )
            nc.vector.scalar_tensor_tensor(
                out=v2m[:], in0=v2[:], scalar=1.0, in1=wt[:],
                op0=OP.mult, op1=OP.mult,
                accum_out=pr[:, 1:2],
            )

        # ---- logits + one fused in-place exp/accumulate pass per chunk
        # S[p, c] = sum_q exp(s_g[p,c] * logit[c*128+p, q] - CMAX)
        S = small.tile([128, PC], f32)
        for c in range(PC):
            pm = psum_mm.tile([128, N1], f32, tag="pm")
            for t0, w in nts:
                for hk in range(2):
                    nc.tensor.matmul(
                        pm[:, t0 : t0 + w],
                        GT[:, hk, c * 128 : (c + 1) * 128],
                        NbT[:, hk, t0 : t0 + w],
                        start=(hk == 0),
                        stop=(hk == 1),
                    )
            if c == min(2, PC - 1):
                # slack slots on ACT: rsqrt of the ep norms
                nc.scalar.activation(
                    out=se[:], in_=ss2[:], func=AF.Ln, bias=eps_t[:, 0:1], scale=1.0
                )
                nc.scalar.activation(
                    out=se[:], in_=se[:], func=AF.Exp, bias=0.0, scale=-0.5
                )
                emit_diag_tail()
            if c == PC - 1:
                # slack slot on ACT just before the last exp: diag exp
                nc.scalar.activation(
                    out=ed[:], in_=diagn[:], func=AF.Exp,
                    bias=cneg_t[:, 0:1], scale=1.0,
                )
            nc.scalar.activation(
                out=pm[:],
                in_=pm[:],
                func=AF.Exp,
                bias=cneg_t[:, 0:1],
                scale=s[:, NC + c : NC + c + 1],
                accum_out=S[:, c : c + 1],
            )

        # ---- tail: t2 = S + corr + ed ; pa = sum(wv * ln(t2))
        t2 = small.tile([128, PC], f32)
        nc.vector.scalar_tensor_tensor(
            out=t2[:], in0=S[:], scalar=corr_t[:, 0:1], in1=ed[:],
            op0=OP.add, op1=OP.add,
        )
        lt = small.tile([128, PC], f32)
        nc.scalar.activation(out=lt[:], in_=t2[:], func=AF.Ln)
        lm = small.tile([128, PC], f32)
        nc.vector.scalar_tensor_tensor(
            out=lm[:], in0=lt[:], scalar=1.0, in1=wt[:],
            op0=OP.mult, op1=OP.mult,
            accum_out=pr[:, 0:1],
        )
        nc.sync.dma_start(out=out[:], in_=pr[:])
    if legalize:
        _legalize_waits(nc, max_waits=1)
    return nc


def _pad_rows(x: np.ndarray, n: int) -> np.ndarray:
    outp = np.zeros((n,) + x.shape[1:], dtype=x.dtype)
    outp[: x.shape[0]] = x
    return outp


def kernel(greek_embeds, english_embeds, labels):
    global LAST_RESULTS, LAST_SHAPES
    g = np.ascontiguousarray(np.asarray(greek_embeds, dtype=np.float32))
    e = np.ascontiguousarray(np.asarray(english_embeds, dtype=np.float32))
    lab = np.asarray(labels)
    B, P, Hh = g.shape
    assert Hh == H and B * 2 == N_CORES

    valid = lab != IGNORE_INDEX
    pos = valid & (lab == 1)
    neg = valid & (lab != 1)
    ok = (valid.sum(-1) >= 2) & pos.any(-1) & neg.any(-1)

    count = int(pos[ok].sum()) if ok.any() else 0
    if count == 0:
        return np.float32(0.0)

    pos_idx = [np.nonzero(pos[b])[0] if ok[b] else np.zeros(0, np.int64) for b in range(B)]
    neg_idx = [np.nonzero(neg[b])[0] if ok[b] else np.zeros(0, np.int64) for b in range(B)]
    halves = [np.array_split(pi, 2) for pi in pos_idx]

    np_max = max(len(halves[b][h]) for b in range(B) for h in range(2))
    nn_max = max(len(ni) for ni in neg_idx)
    P1 = max(128, ((np_max + 127) // 128) * 128)
    N1 = max(128, ((nn_max + 127) // 128) * 128)

    E15 = np.float32(np.exp(np.float32(-CMAX)))
    in_maps = []
    for core in range(N_CORES):
        bb, hf = core // 2, core % 2
        p_idx = halves[bb][hf]
        n_idx = neg_idx[bb]
        w = np.zeros(P1, np.float32)
        w[: len(p_idx)] = 1.0
        gp_pad = _pad_rows(g[bb][p_idx].astype(ml_dtypes.bfloat16), P1)
        in_maps.append(
            {
                "gp": gp_pad,
                "gpt": np.ascontiguousarray(gp_pad.T).reshape(2, 128, P1),
                "ep": _pad_rows(e[bb][p_idx].astype(ml_dtypes.bfloat16), P1),
                "en": _pad_rows(e[bb][n_idx].astype(ml_dtypes.bfloat16), N1),
                "wv": w,
                "corr": np.array([[-(N1 - len(n_idx)) * float(E15)]], np.float32),
            }
        )

    LAST_SHAPES = (P1, N1, dict(in_maps[0]))
    nc = _build_program(P1, N1)
    res = run_bass_kernel_spmd(nc, in_maps, list(range(N_CORES)), trace=TRACE)
    LAST_RESULTS = res
    total = sum(float(r["out"].sum()) for r in res.results)
    return np.float32(total / count)


# revision 13
# speedup vs baseline: 1.1218x; 1.0660x over previous
"""Contrastive loss kernel for Trainium2 (8 NeuronCores, Bass/Tile).

Strategy
--------
Only rows with label==1 (pos) contribute losses, and only columns with
label==0 (neg) plus the diagonal enter each row's logsumexp.  The host
computes the tiny index sets from `labels`, then each of the 8 cores
(2 per batch) receives:
  gp : its half of the batch's positive greek rows, row-major [P1,256]
  gpt: the same rows pre-transposed on host [2,128,P1] (H on partitions)
  ep : english rows at the same indices (diag term)    [P1,256]
  en : all negative english rows of the batch          [N1,256]
all bf16, zero-padded to uniform compile-time shapes (P1, N1).  N2 <= N1
is the exact used width of the neg axis (the matmul/exp only touch N2
columns; the transposes run on full 128-row chunks).

Device pipeline: row sums-of-squares via fused square+accumulate ops
(DVE scalar_tensor_tensor + a few Scalar-engine Square activations,
which share the Ln/Exp table so no reload), inverse norms via per-piece
Ln / Exp(scale=-0.5) pairs.  The greek scale (1/T folded via a T^2
factor in its squares) is applied as the per-partition `scale` operand
of the exp pass, so raw host-transposed gpt feeds the matmul directly.
English rows are scaled with 4x-mode tensor_scalar ops (DVE/Pool
split), PE-transposed, and copied to SBUF.  Logits accumulate in PSUM
(bf16 matmul); one in-place exp(s_g*x - 15) pass with accumulate per
128-row pos chunk produces the negative sums.  Zero-padded en columns
give exactly 0 logits; their exp(-15) mass is removed by an exact
host-computed correction.  The diag path (second gp piece, ep norms,
dot products) is pushed into the exp phase with scheduler wait hints.
Per-row loss splits into wv*ln(S+corr+e^(d-15)) + wv*(15-d); both
partial sums reduce on-device; the host sums 8x128x2 partials / count.
"""

import sys

if "/opt/trn_rl_repo" not in sys.path:
    sys.path.insert(0, "/opt/trn_rl_repo")

from contextlib import ExitStack

import ml_dtypes
import numpy as np

import concourse.bass as bass
import concourse.tile as tile
from concourse import mybir
from concourse.bass_utils import run_bass_kernel_spmd
from concourse.masks import make_identity

TEMPERATURE = 0.07
IGNORE_INDEX = -100
CMAX = 15.0
H = 256
N_CORES = 8

# Stash of the most recent BassKernelResults + shapes (for test harness timing).
LAST_RESULTS = None
LAST_SHAPES = None
TRACE = False


def _legalize_waits(nc: bass.Bass, max_waits: int = 1) -> None:
    """This container's walrus accepts at most one sync-wait per instruction
    (ACT structs especially); Tile can emit several.  Split the excess onto
    same-engine NoOps placed immediately before the instruction."""
    for bb in nc.main_func.blocks:
        new = []
        for ins in bb.instructions:
            si = ins.sync_info
            if si is not None and si.on_wait and len(si.on_wait) > max_waits:
                waits = list(si.on_wait)
                extra, keep = waits[:-max_waits], waits[-max_waits:]
                for i in range(0, len(extra), max_waits):
                    new.append(
                        mybir.InstNoOp(
                            name=nc.get_next_instruction_name(),
                            engine=ins.engine,
                            ins=[],
                            outs=[],
                            sync_info=mybir.SyncInfo(
                                on_wait=extra[i : i + max_waits], on_update=[]
                            ),
                            bass_nofuse=True,
                        )
                    )
                ins.sync_info = mybir.SyncInfo(
                    on_wait=keep, on_update=list(si.on_update or [])
                )
            new.append(ins)
        bb.instructions[:] = new


def _build_program(P1: int, N1: int, N2: int, legalize: bool = True) -> bass.Bass:
    """One SPMD program: shapes P1 (pos rows) / N1 (padded neg rows) / N2
    (used neg width) are uniform across cores; data differs via in_maps."""
    PC = P1 // 128
    NC = N1 // 128
    GA = min(2, PC)  # early greek piece (chunks [0:GA])
    assert N2 <= N1 <= 1536 and P1 <= 1664
    f32 = mybir.dt.float32
    bf16 = mybir.dt.bfloat16
    OP = mybir.AluOpType
    AF = mybir.ActivationFunctionType

    # en DMA pieces: first two on SP, rest on Pool SWDGE
    bounds = [0, min(3, NC), min(6, NC), min(9, NC), NC]
    pieces = [
        (bounds[i], bounds[i + 1]) for i in range(4) if bounds[i + 1] > bounds[i]
    ]
    piece_q = [nc_q for nc_q, (lo, hi) in zip([0, 0, 1, 1], pieces)]
    # ss layout: en chunks [0:NC], greek-A [NC:NC+GA]
    # ACT Square handles the gA chunks + a couple of en chunks.
    act_sq_en = {3, 4} if NC > 4 else set()
    # 512-wide matmul tiles over the used width N2
    nts = []
    c0 = 0
    while c0 < N2:
        w = min(512, N2 - c0)
        nts.append((c0, w))
        c0 += w

    nc = bass.Bass()
    gp = nc.dram_tensor("gp", [P1, H], bf16, kind="ExternalInput")
    gpt = nc.dram_tensor("gpt", [2, 128, P1], bf16, kind="ExternalInput")
    ep = nc.dram_tensor("ep", [P1, H], bf16, kind="ExternalInput")
    en = nc.dram_tensor("en", [N1, H], bf16, kind="ExternalInput")
    wv = nc.dram_tensor("wv", [P1], f32, kind="ExternalInput")
    corr = nc.dram_tensor("corr", [1, 1], f32, kind="ExternalInput")
    out = nc.dram_tensor("out", [128, 2], f32, kind="ExternalOutput")

    with tile.TileContext(nc) as tc, ExitStack() as ctx:
        persist = ctx.enter_context(tc.tile_pool(name="persist", bufs=1))
        small = ctx.enter_context(tc.tile_pool(name="small", bufs=1))
        scratch = ctx.enter_context(tc.tile_pool(name="scratch", bufs=4))
        psum_tp = ctx.enter_context(tc.tile_pool(name="psum_tp", bufs=2, space="PSUM"))
        psum_mm = ctx.enter_context(tc.tile_pool(name="psum_mm", bufs=2, space="PSUM"))

        # ---- constants (gpsimd), highest priority so the PE warmup can
        # anchor the p-state ramp immediately.
        with tc.high_priority():
            cneg_t = small.tile([128, 1], f32)
            nc.gpsimd.memset(cneg_t[:], -CMAX)
            eps_t = small.tile([128, 1], f32)
            nc.gpsimd.memset(eps_t[:], 1e-24)
            ident = small.tile([128, 128], bf16)
            make_identity(nc, ident[:])
            ptw = psum_tp.tile([128, 384], bf16, tag="pt")
            nc.tensor.transpose(ptw[:, 0:128], ident[:], ident[:])

        # ---- DMA loads ---------------------------------------------------
        # ACT queue: early greek piece, then the table preload.
        GfA = persist.tile([128, GA, H], bf16, name="gfa")
        gp_r = gp[:].rearrange("(c p) h -> p c h", p=128)
        nc.scalar.dma_start(out=GfA[:], in_=gp_r[:, 0:GA, :])
        dummy = small.tile([128, 1], f32)
        nc.scalar.activation(
            out=dummy[:], in_=eps_t[:], func=AF.Ln, bias=eps_t[:, 0:1], scale=1.0
        )

        en_r = en[:].rearrange("(c p) h -> p c h", p=128)
        EnP = []
        for qi, (lo, hi) in zip(piece_q, pieces):
            t = persist.tile([128, hi - lo, H], bf16, tag=f"en{lo}", name=f"en{lo}")
            eng = [nc.sync, nc.gpsimd][qi]
            eng.dma_start(out=t[:], in_=en_r[:, lo:hi, :])
            EnP.append(t)

        def en_chunk(c):
            for (lo, hi), t in zip(pieces, EnP):
                if lo <= c < hi:
                    return t[:, c - lo, :]
            raise AssertionError

        GfB = None
        if PC > GA:
            GfB = persist.tile([128, PC - GA, H], bf16, name="gfb")
            nc.sync.dma_start(out=GfB[:], in_=gp_r[:, GA:PC, :])
        wt = small.tile([128, PC], f32)
        nc.sync.dma_start(out=wt[:], in_=wv[:].rearrange("(c p) -> p c", p=128))
        corr_t = small.tile([128, 1], f32)
        nc.sync.dma_start(out=corr_t[:], in_=corr[:].to_broadcast([128, 1]))

        GT = persist.tile([128, 2, P1], bf16)
        nc.gpsimd.dma_start(out=GT[:], in_=gpt[:].rearrange("k p j -> p k j"))
        Ef = persist.tile([128, PC, H], bf16)
        nc.gpsimd.dma_start(out=Ef[:], in_=ep[:].rearrange("(c p) h -> p c h", p=128))

        def gf_chunk(c):
            if c < GA:
                return GfA[:, c, :]
            return GfB[:, c - GA, :]

        # ---- row sums of squares ----------------------------------------
        ss = small.tile([128, NC + GA], f32)

        def sq_dve(src, ss_t, col, scalar=1.0):
            sq = scratch.tile([128, H], bf16, tag="sq")
            nc.vector.scalar_tensor_tensor(
                out=sq[:], in0=src, scalar=scalar, in1=src,
                op0=OP.mult, op1=OP.mult,
                accum_out=ss_t[:, col : col + 1],
            )

        def sq_act(src, ss_t, col, scale=1.0):
            sq = scratch.tile([128, H], bf16, tag="sq")
            nc.scalar.activation(
                out=sq[:], in_=src, func=AF.Square, bias=0.0, scale=scale,
                accum_out=ss_t[:, col : col + 1],
            )

        for c in range(GA):
            sq_act(GfA[:, c, :], ss, NC + c, scale=float(TEMPERATURE))
        for c in range(NC):
            if c in act_sq_en:
                sq_act(en_chunk(c), ss, c)
            else:
                sq_dve(en_chunk(c), ss, c)

        # ---- inverse norms per piece: s = (ss+eps)^-0.5 via Ln + Exp
        s = small.tile([128, NC + GA], f32)

        def rsqrt(lo, hi):
            nc.scalar.activation(
                out=s[:, lo:hi], in_=ss[:, lo:hi], func=AF.Ln,
                bias=eps_t[:, 0:1], scale=1.0,
            )
            nc.scalar.activation(
                out=s[:, lo:hi], in_=s[:, lo:hi], func=AF.Exp, bias=0.0, scale=-0.5
            )

        rsqrt(NC, NC + GA)  # greek piece A (exp scales for chunks < GA)
        for lo, hi in pieces[:-1]:
            rsqrt(lo, hi)
        rsqrt(pieces[-1][0], pieces[-1][1])

        # ---- scale en rows (4x-mode DVE / Pool split), transpose, copy --
        Enb = persist.tile([128, NC, H], bf16)
        NbT = persist.tile([128, 2, N1], bf16)
        for pi, (lo, hi) in enumerate(pieces):
            for c in range(lo, hi):
                eng = nc.vector if c % 2 == 0 else nc.gpsimd
                eng.tensor_scalar_mul(Enb[:, c, :], en_chunk(c), s[:, c : c + 1])
            gn = hi - lo
            for hk in range(2):
                pt = psum_tp.tile([128, 384], bf16, tag="pt")
                for j in range(gn):
                    nc.tensor.transpose(
                        pt[:, j * 128 : (j + 1) * 128],
                        Enb[:, lo + j, hk * 128 : (hk + 1) * 128],
                        ident[:],
                    )
                nc.vector.tensor_copy(
                    out=NbT[:, hk, lo * 128 : hi * 128], in_=pt[:, : gn * 128]
                )

        # ---- diag path: emitted with wait hints so it fills the exp-phase
        # slack instead of stealing the pre-exp DVE/Pool lanes.
        ssl = small.tile([128, (PC - GA) + PC], f32)  # [g_late | ep]
        dot = small.tile([128, PC], f32)
        with tc.tile_wait_until(0.006):
            for c in range(GA, PC):
                sq_dve(gf_chunk(c), ssl, c - GA, scalar=float(TEMPERATURE**2))
            for c in range(PC):
                sq_dve(Ef[:, c, :], ssl, (PC - GA) + c)
        with tc.tile_wait_until(0.008):
            for c in range(PC):
                dsq = scratch.tile([128, H], bf16, tag="sq")
                nc.vector.scalar_tensor_tensor(
                    out=dsq[:], in0=gf_chunk(c), scalar=1.0, in1=Ef[:, c, :],
                    op0=OP.mult, op1=OP.mult,
                    accum_out=dot[:, c : c + 1],
                )
        sl = small.tile([128, (PC - GA) + PC], f32)
        se = sl[:, PC - GA :]
        diagn = small.tile([128, PC], f32)
        ed = small.tile([128, PC], f32)
        v2 = small.tile([128, PC], f32)
        pr = small.tile([128, 2], f32)
        v2m = small.tile([128, PC], f32)

        def emit_diag_tail():
            sg = small.tile([128, PC], f32)
            nc.vector.tensor_copy(out=sg[:, 0:GA], in_=s[:, NC : NC + GA])
            if PC > GA:
                nc.vector.tensor_copy(out=sg[:, GA:], in_=sl[:, 0 : PC - GA])
            nc.vector.tensor_tensor(out=diagn[:], in0=dot[:], in1=sg[:], op=OP.mult)
            nc.vector.tensor_tensor(out=diagn[:], in0=diagn[:], in1=se, op=OP.mult)
            # v2 = 15 - diagn ; pv = sum(wv * v2)
            nc.vector.tensor_scalar(
                out=v2[:], in0=diagn[:], scalar1=-1.0, scalar2=CMAX,
                op0=OP.mult, op1=OP.add,
            )
            nc.vector.scalar_tensor_tensor(
                out=v2m[:], in0=v2[:], scalar=1.0, in1=wt[:],
                op0=OP.mult, op1=OP.mult,
                accum_out=pr[:, 1:2],
            )

        # ---- logits + one fused in-place exp/accumulate pass per chunk --
        # S[p, c] = sum_{q<N2} exp(s_g[p,c] * logit[c*128+p, q] - CMAX)
        S = small.tile([128, PC], f32)
        for c in range(PC):
            pm = psum_mm.tile([128, N2], f32, tag="pm")
            for t0, w in nts:
                for hk in range(2):
                    nc.tensor.matmul(
                        pm[:, t0 : t0 + w],
                        GT[:, hk, c * 128 : (c + 1) * 128],
                        NbT[:, hk, t0 : t0 + w],
                        start=(hk == 0),
                        stop=(hk == 1),
                    )
            if c == min(2, PC - 1):
                # one inserted rsqrt pair: late greek scales + ep norms
                nc.scalar.activation(
                    out=sl[:], in_=ssl[:], func=AF.Ln, bias=eps_t[:, 0:1], scale=1.0
                )
                nc.scalar.activation(
                    out=sl[:], in_=sl[:], func=AF.Exp, bias=0.0, scale=-0.5
                )
                emit_diag_tail()
            if c == PC - 1:
                nc.scalar.activation(
                    out=ed[:], in_=diagn[:], func=AF.Exp,
                    bias=cneg_t[:, 0:1], scale=1.0,
                )
            scale_ap = s[:, NC + c : NC + c + 1] if c < GA else sl[:, c - GA : c - GA + 1]
            nc.scalar.activation(
                out=pm[:],
                in_=pm[:],
                func=AF.Exp,
                bias=cneg_t[:, 0:1],
                scale=scale_ap,
                accum_out=S[:, c : c + 1],
            )

        # ---- tail: t2 = S + corr + ed ; pa = sum(wv * ln(t2))
        t2 = small.tile([128, PC], f32)
        nc.vector.scalar_tensor_tensor(
            out=t2[:], in0=S[:], scalar=corr_t[:, 0:1], in1=ed[:],
            op0=OP.add, op1=OP.add,
        )
        lt = small.tile([128, PC], f32)
        nc.scalar.activation(out=lt[:], in_=t2[:], func=AF.Ln)
        lm = small.tile([128, PC], f32)
        nc.vector.scalar_tensor_tensor(
            out=lm[:], in0=lt[:], scalar=1.0, in1=wt[:],
            op0=OP.mult, op1=OP.mult,
            accum_out=pr[:, 0:1],
        )
        nc.sync.dma_start(out=out[:], in_=pr[:])
    if legalize:
        _legalize_waits(nc, max_waits=1)
    return nc


def _pad_rows(x: np.ndarray, n: int) -> np.ndarray:
    outp = np.zeros((n,) + x.shape[1:], dtype=x.dtype)
    outp[: x.shape[0]] = x
    return outp


def kernel(greek_embeds, english_embeds, labels):
    global LAST_RESULTS, LAST_SHAPES
    g = np.ascontiguousarray(np.asarray(greek_embeds, dtype=np.float32))
    e = np.ascontiguousarray(np.asarray(english_embeds, dtype=np.float32))
    lab = np.asarray(labels)
    B, P, Hh = g.shape
    assert Hh == H and B * 2 == N_CORES

    valid = lab != IGNORE_INDEX
    pos = valid & (lab == 1)
    neg = valid & (lab != 1)
    ok = (valid.sum(-1) >= 2) & pos.any(-1) & neg.any(-1)

    count = int(pos[ok].sum()) if ok.any() else 0
    if count == 0:
        return np.float32(0.0)

    pos_idx = [np.nonzero(pos[b])[0] if ok[b] else np.zeros(0, np.int64) for b in range(B)]
    neg_idx = [np.nonzero(neg[b])[0] if ok[b] else np.zeros(0, np.int64) for b in range(B)]
    halves = [np.array_split(pi, 2) for pi in pos_idx]

    np_max = max(len(halves[b][h]) for b in range(B) for h in range(2))
    nn_max = max(len(ni) for ni in neg_idx)
    P1 = max(128, ((np_max + 127) // 128) * 128)
    N2 = max(16, ((nn_max + 15) // 16) * 16)
    N1 = max(128, ((N2 + 127) // 128) * 128)

    E15 = np.float32(np.exp(np.float32(-CMAX)))
    in_maps = []
    for core in range(N_CORES):
        bb, hf = core // 2, core % 2
        p_idx = halves[bb][hf]
        n_idx = neg_idx[bb]
        w = np.zeros(P1, np.float32)
        w[: len(p_idx)] = 1.0
        gp_pad = _pad_rows(g[bb][p_idx].astype(ml_dtypes.bfloat16), P1)
        in_maps.append(
            {
                "gp": gp_pad,
                "gpt": np.ascontiguousarray(gp_pad.T).reshape(2, 128, P1),
                "ep": _pad_rows(e[bb][p_idx].astype(ml_dtypes.bfloat16), P1),
                "en": _pad_rows(e[bb][n_idx].astype(ml_dtypes.bfloat16), N1),
                "wv": w,
                "corr": np.array([[-(N2 - len(n_idx)) * float(E15)]], np.float32),
            }
        )

    LAST_SHAPES = (P1, N1, N2, dict(in_maps[0]))
    nc = _build_program(P1, N1, N2)
    res = run_bass_kernel_spmd(nc, in_maps, list(range(N_CORES)), trace=TRACE)
    LAST_RESULTS = res
    total = sum(float(r["out"].sum()) for r in res.results)
    return np.float32(total / count)
